# revision 80
# baseline (speedup 1.0000x reference)
"""Single-head attention (B=4, T=4096, E=1024, D=64) on 8 TRN2 NeuronCores.

Sharding: data-parallel over (batch, query-half): core c -> batch c//2,
query half c%2.  Each core receives the full x[b] pre-transposed on the
host, with rows rotated so its OWN query half always occupies columns
0:2048 (keeps the SPMD graph identical across cores; attention is
permutation-invariant over keys).

Per-core pipeline (score-chain matmuls in fp16):
  1. Projections (PE, stationary weights) per 512-token chunk:
     own chunks also run [Wq|Wq] (Q^T duplicated to BOTH partition
     halves -- output duplication is free, matmul cost = moving cols);
     even chunks run [Wk|Wv/8] (K^T -> rows 0:64, V^T -> rows 64:128),
     odd chunks run [Wv/8|Wk] (V^T -> rows 0:64, K^T -> rows 64:128).
     The 1/sqrt(D) is folded into Wv.
  2. V^T -> V' = [V | ones] strips via PE-transpose (the ones column
     makes P @ V' also emit softmax row sums).
  3. Scores are ROW-PACKED on the PE: pair p = (tile from an even
     chunk at PE rows 0:63, tile from an odd chunk at rows 64:127).
     The two 64-contraction matmuls run concurrently on disjoint
     row-groups (tile_position comes from the operands' base
     partitions), doubling score throughput.  exp on ScalarE
     (PSUM -> SBUF bf16), O^T += V'.T @ P^T into [65, 1024] PSUM.
  4. O^T (unnormalized, with the sums row) is DMA'd straight to DRAM;
     the host does the transpose + divide (free).

Softmax runs without max-subtraction: scores are ~N(0, 64) so |s| << 88
(fp32 exp overflow); the reference's max-subtraction is a no-op.
"""

import os
import sys
from collections import deque

import numpy as np

_TRN_REPO = "/opt/trn_rl_repo"
if _TRN_REPO not in sys.path:
    sys.path.insert(0, _TRN_REPO)

import concourse.bass as bass  # noqa: E402
import concourse.mybir as mybir  # noqa: E402
import concourse.tile as tile  # noqa: E402
from concourse import bacc  # noqa: E402
from concourse.bass_utils import run_bass_kernel_spmd  # noqa: E402

F32 = mybir.dt.float32
F16 = mybir.dt.float16
BF16 = mybir.dt.bfloat16

B, T, E, D = 4, 4096, 1024, 64
TH = T // 2  # queries per core
NCORES = 8
QPASS = 1024  # queries per PSUM pass
NMM = 512  # matmul moving free dim (one fp32 PSUM bank)
NKT = T // 128  # 32 key tiles of 128
NPAIR = NKT // 2  # 16 row-packed tile pairs
EK = E // 128  # 8 contraction tiles for projections
CHUNK = 512  # projection chunk (tokens)
NCH = T // CHUNK  # 8 chunks

SCORE_DT = F16
SCORE_NP = np.float16
PV_DT = BF16  # P = exp(S) reaches ~1e20: needs bf16 range


def _tile_a(p):  # even-chunk tile of pair p (PE rows 0:64)
    return 8 * (p // 4) + p % 4


def _tile_b(p):  # odd-chunk tile of pair p (PE rows 64:128)
    return _tile_a(p) + 4


def _build_nc() -> bass.Bass:
    nc = bacc.Bacc(
        "TRN2",
        target_bir_lowering=False,
        debug=False,
        num_devices=NCORES,
    )
    xT_d = nc.dram_tensor("xT", [E, T], SCORE_DT, kind="ExternalInput")
    # weights arrive partition-major ([128, E], e-blocks along the free
    # dim) so the load is 128 x 2KB descriptors instead of 1024 x 256B
    wqq_d = nc.dram_tensor("wqq", [128, E], SCORE_DT, kind="ExternalInput")
    wkv_d = nc.dram_tensor("wkv", [128, E], SCORE_DT, kind="ExternalInput")
    wvk_d = nc.dram_tensor("wvk", [128, E], SCORE_DT, kind="ExternalInput")
    out_d = nc.dram_tensor("outT", [D + 1, TH], F32, kind="ExternalOutput")

    with tile.TileContext(nc) as tc:
        with (
            tc.tile_pool(name="consts", bufs=1) as consts,
            tc.tile_pool(name="big", bufs=1) as big,
            tc.tile_pool(name="pt", bufs=8) as ptpool,
            tc.tile_pool(name="osb", bufs=2) as osbpool,
            tc.tile_pool(name="auxp", bufs=2, space="PSUM") as auxp,
            tc.tile_pool(name="stp", bufs=2, space="PSUM") as stp,
            tc.tile_pool(name="otp", bufs=1, space="PSUM") as otp,
        ):
            # identity for the V^T -> V PE-transposes, on both partition
            # halves (even chunks park V^T at rows 64:128, odd at 0:64);
            # emitted first so the gpsimd queue is clear for xT loads
            identB = consts.tile([128, 64], PV_DT, tag="identB")
            from concourse.masks import make_identity

            nc.gpsimd.memset(identB[:], 0.0)
            make_identity(nc, identB[0:64, 0:64], nomemset=True)
            make_identity(nc, identB[64:128, 0:64], nomemset=True)

            # weights go on the scalar queue: it is idle this early, and
            # these three transfers drain long before the first exp
            wqq = consts.tile([128, E], SCORE_DT, tag="wqq")
            wkv = consts.tile([128, E], SCORE_DT, tag="wkv")
            wvk = consts.tile([128, E], SCORE_DT, tag="wvk")
            for w_sb, w_d in ((wqq, wqq_d), (wkv, wkv_d), (wvk, wvk_d)):
                nc.scalar.dma_start(w_sb[:], w_d[:, :])

            # V' strip: 32 tiles of [128 keys, 64 V cols + 1 ones col],
            # padded to stride 128.  Only the ones columns need init
            # (cols 0:64 are overwritten, 65:128 never read).
            vprime = consts.tile([128, NKT * 128], PV_DT, tag="vprime")
            nc.vector.memset(
                vprime[:].rearrange("p (k c) -> p k c", c=128)[:, :, 64:65], 1.0
            )

            # (no explicit exp-table prewarm: the auto-inserted table
            # load before the first real exp overlaps the proj phase; an
            # early ACTIVATE gets hoisted ahead of the weight DMAs on
            # the scalar queue and delays the first projection)

            # HAM warmup: keep the PE busy from t~0 until the first
            # projection chunk lands so real work runs at 2.4 GHz
            # warm PSUM lives in the otp pool slot (free until pass 0's
            # accumulator is allocated) so fillers never clobber the
            # rotating aux slots used by in-flight projections
            warm = consts.tile([128, 256], SCORE_DT, tag="warm")
            nc.vector.memset(warm[:], 0.0)
            wps = otp.tile([128, 256], F32, tag="ot", name="wps")
            # 16 x 256 cols ~= 3.4us at cold clock: bridges from t~8us to
            # the first projection AND satisfies the HAM SHORT window so
            # the projections run at 2.4 GHz from the start
            for _ in range(16):
                nc.tensor.matmul(wps[:], warm[:, 0:128], warm[:], start=True, stop=True)

            q2 = big.tile([128, TH], SCORE_DT, tag="q2")  # Q^T on both halves
            k2 = big.tile([128, NPAIR * 128], SCORE_DT, tag="k2")
            vt = big.tile([128, T], PV_DT, tag="vt")

            # ---- x^T loads: chunk-major so chunk c unlocks after 8 tiles ----
            # the scalar queue is deliberately NOT used for input DMAs:
            # its semaphore waits would block the ACTIVATEs (exp) queued
            # behind them on the same engine
            xts = [None] * NCH
            engs = (nc.sync, nc.gpsimd)
            ei = 0
            for c in range(NCH):
                xt = big.tile([128, EK * CHUNK], SCORE_DT, tag=f"xt{c}", name=f"xt{c}")
                for e in range(EK):
                    engs[ei % 2].dma_start(
                        xt[:, e * CHUNK : (e + 1) * CHUNK],
                        xT_d[e * 128 : (e + 1) * 128, c * CHUNK : (c + 1) * CHUNK],
                    )
                    ei += 1
                xts[c] = xt

            # ---- projection emitters ----
            def emit_proj_Q(c):
                xt = xts[c]
                pa = auxp.tile([128, CHUNK], F32, tag="aux", name=f"pa{c}")
                for e in range(EK):
                    nc.tensor.matmul(
                        pa[:],
                        wqq[:, e * 128 : (e + 1) * 128],
                        xt[:, e * CHUNK : (e + 1) * CHUNK],
                        start=(e == 0),
                        stop=(e == EK - 1),
                    )
                nc.vector.tensor_copy(q2[:, c * CHUNK : (c + 1) * CHUNK], pa[:])

            def emit_proj_K(c):
                even = c % 2 == 0
                xt = xts[c]
                wkvx = wkv if even else wvk
                krows, vrows = (
                    (slice(0, 64), slice(64, 128))
                    if even
                    else (slice(64, 128), slice(0, 64))
                )
                pb = auxp.tile([128, CHUNK], F32, tag="aux", name=f"pb{c}")
                for e in range(EK):
                    nc.tensor.matmul(
                        pb[:],
                        wkvx[:, e * 128 : (e + 1) * 128],
                        xt[:, e * CHUNK : (e + 1) * CHUNK],
                        start=(e == 0),
                        stop=(e == EK - 1),
                    )
                nc.vector.tensor_copy(
                    k2[krows, (c // 2) * CHUNK : (c // 2 + 1) * CHUNK],
                    pb[krows, :],
                )
                nc.vector.tensor_copy(
                    vt[vrows, c * CHUNK : (c + 1) * CHUNK], pb[vrows, :]
                )
                for j in range(4):
                    kb = 4 * c + j
                    tp = auxp.tile([128, D], PV_DT, tag="aux", name=f"tp{kb}")
                    nc.tensor.transpose(
                        tp[:],
                        vt[vrows, kb * 128 : (kb + 1) * 128],
                        identB[vrows, 0:64],
                    )
                    nc.vector.tensor_copy(
                        vprime[:, kb * 128 : kb * 128 + D], tp[:]
                    )

            # chunks 0,1: Q and K sweeps interleaved per e-tile so the
            # projections trail the DMA arrivals by ~2 matmuls
            for c in (0, 1):
                xt = xts[c]
                wkvx = wkv if c % 2 == 0 else wvk
                # Q accumulators borrow the (still unused) score-pool
                # slots so all four head accumulators coexist without
                # PSUM rotation conflicts
                pa = stp.tile([128, CHUNK], F32, tag="st", name=f"pa{c}")
                pb = auxp.tile([128, CHUNK], F32, tag="aux", name=f"pb{c}")
                for e in range(EK):
                    nc.tensor.matmul(
                        pa[:],
                        wqq[:, e * 128 : (e + 1) * 128],
                        xt[:, e * CHUNK : (e + 1) * CHUNK],
                        start=(e == 0),
                        stop=(e == EK - 1),
                    )
                    nc.tensor.matmul(
                        pb[:],
                        wkvx[:, e * 128 : (e + 1) * 128],
                        xt[:, e * CHUNK : (e + 1) * CHUNK],
                        start=(e == 0),
                        stop=(e == EK - 1),
                    )

                nc.vector.tensor_copy(q2[:, c * CHUNK : (c + 1) * CHUNK], pa[:])
                krows, vrows = (
                    (slice(0, 64), slice(64, 128))
                    if c % 2 == 0
                    else (slice(64, 128), slice(0, 64))
                )
                nc.vector.tensor_copy(
                    k2[krows, (c // 2) * CHUNK : (c // 2 + 1) * CHUNK], pb[krows, :]
                )
                nc.vector.tensor_copy(
                    vt[vrows, c * CHUNK : (c + 1) * CHUNK], pb[vrows, :]
                )
                for j in range(4):
                    kb = 4 * c + j
                    tp = auxp.tile([128, D], PV_DT, tag="aux", name=f"tp{kb}")
                    nc.tensor.transpose(
                        tp[:],
                        vt[vrows, kb * 128 : (kb + 1) * 128],
                        identB[vrows, 0:64],
                    )
                    nc.vector.tensor_copy(vprime[:, kb * 128 : kb * 128 + D], tp[:])

            # remaining projections: full K units (dense PE blocks keep
            # the HAM clock warm), pair 0 left clean so the first exps
            # fire as early as possible; Q-sweeps of chunks 2,3 (pass-1
            # queries) deferred to the end of pass 0
            chunk_sched = {
                1: [("K", 2)],
                2: [("K", 3)],
                4: [("K", 4)],
                5: [("K", 5)],
                8: [("K", 6)],
                9: [("K", 7)],
                12: [("Q", 2)],
                13: [("Q", 3)],
            }

            # ---- attention passes ----
            for qp in range(TH // QPASS):
                q0 = qp * QPASS
                ot = otp.tile([D + 1, QPASS], F32, tag="ot")
                pending_av = deque()
                n_av = [0]

                def emit_av(avpt, avkt):
                    for qc in range(0, QPASS, NMM):
                        nc.tensor.matmul(
                            ot[:, qc : qc + NMM],
                            vprime[:, avkt * 128 : avkt * 128 + D + 1],
                            avpt[:, qc : qc + NMM],
                            start=(n_av[0] == 0),
                            stop=(n_av[0] == NKT - 1),
                        )
                    n_av[0] += 1

                for p in range(NPAIR):
                    ka, kb = _tile_a(p), _tile_b(p)
                    st_e = stp.tile([128, QPASS], F32, tag="st", name=f"se{qp}_{p}")
                    st_o = stp.tile([128, QPASS], F32, tag="st", name=f"so{qp}_{p}")
                    for qc in range(0, QPASS, NMM):
                        nc.tensor.matmul(
                            st_e[:, qc : qc + NMM],
                            k2[0:64, p * 128 : (p + 1) * 128],
                            q2[0:64, q0 + qc : q0 + qc + NMM],
                            start=True,
                            stop=True,
                        )
                        nc.tensor.matmul(
                            st_o[:, qc : qc + NMM],
                            k2[64:128, p * 128 : (p + 1) * 128],
                            q2[64:128, q0 + qc : q0 + qc + NMM],
                            start=True,
                            stop=True,
                        )
                    pt_e = ptpool.tile([128, QPASS], PV_DT, tag="pt", name=f"pe{qp}_{p}")
                    nc.scalar.activation(
                        pt_e[:], st_e[:], mybir.ActivationFunctionType.Exp
                    )
                    pt_o = ptpool.tile([128, QPASS], PV_DT, tag="pt", name=f"po{qp}_{p}")
                    nc.scalar.activation(
                        pt_o[:], st_o[:], mybir.ActivationFunctionType.Exp
                    )
                    pending_av.append((pt_e, ka))
                    pending_av.append((pt_o, kb))
                    # shallower backlog near the pass end shortens the tail
                    depth = 3 if p < NPAIR - 2 else (1 if p < NPAIR - 1 else 0)
                    while len(pending_av) > depth:
                        emit_av(*pending_av.popleft())
                    # projection units go AFTER the pair's scores so the
                    # exp chain is never pushed out by projection matmuls
                    if qp == 0 and p in chunk_sched:
                        for kind, c in chunk_sched[p]:
                            (emit_proj_K if kind == "K" else emit_proj_Q)(c)
                while pending_av:
                    emit_av(*pending_av.popleft())

                # ship the unnormalized O^T (plus the sums row) straight
                # out; the host does transpose + divide.  Two halves so
                # the copy of half 0 overlaps the final AVs of half 1.
                osb = osbpool.tile([D + 1, QPASS], F32, tag="osb")
                for h in (0, 1):
                    sl = slice(h * (QPASS // 2), (h + 1) * (QPASS // 2))
                    nc.vector.tensor_copy(osb[:, sl], ot[:, sl])
                    nc.sync.dma_start(out_d[:, q0 + h * (QPASS // 2) : q0 + (h + 1) * (QPASS // 2)], osb[:, sl])

    _elide_redundant_ldweights(nc)
    nc.compile()
    return nc


def _elide_redundant_ldweights(nc):
    """Drop an InstLdweights whose stationary AP is identical to the
    previous one with only plain matmuls between (the legalizer emits one
    load per matmul; consecutive same-weights loads are dead)."""
    removed = 0
    for blk in nc.main_func.blocks:
        last_key = {}  # row-group (base partition span) -> AP key
        keep = []
        for inst in blk.instructions:
            if isinstance(inst, mybir.InstLdweights):
                si = inst.sync_info
                clean = si is None or (not si.on_wait and not si.on_update)
                ap = inst.ins[0]
                key = repr(ap)
                bap = getattr(ap, "bass_ap", None)
                part0 = psz = None
                if bap is not None:
                    try:
                        part0 = bap.base_partition()
                        psz = bap.partition_size()
                    except Exception:
                        part0 = psz = None
                grp = (part0, psz)
                full = psz is None or part0 is None or psz > 64
                if clean and part0 is not None and last_key.get(grp) == key:
                    removed += 1
                    continue
                if full:
                    last_key.clear()
                    if part0 is not None:
                        last_key[grp] = key
                else:
                    # a load into one row-group leaves other groups intact
                    last_key = {
                        g: k
                        for g, k in last_key.items()
                        if g[0] + (g[1] or 128) <= part0
                        or part0 + (psz or 128) <= g[0]
                    }
                    last_key[grp] = key
                keep.append(inst)
                continue
            if getattr(inst, "engine", None) == mybir.EngineType.PE:
                if not (
                    isinstance(inst, mybir.InstMatmult)
                    and not getattr(inst, "is_transpose", False)
                ):
                    last_key = {}
            keep.append(inst)
        blk.instructions[:] = keep
    return removed


_NC_CACHE = None
LAST_RESULT = None


def _get_nc():
    global _NC_CACHE
    if _NC_CACHE is None:
        _NC_CACHE = _build_nc()
    return _NC_CACHE


def make_in_maps(x, Wq, Wk, Wv):
    x = np.asarray(x, dtype=np.float32)
    Wq = np.asarray(Wq, dtype=np.float32)
    Wk = np.asarray(Wk, dtype=np.float32)
    Wv = np.asarray(Wv, dtype=np.float32)
    wv8 = Wv / np.sqrt(np.float32(D))

    def pack_w(wcat):  # [E, 128] -> [128, E] partition-major, e-blocks
        return np.ascontiguousarray(
            wcat.reshape(EK, 128, 128).transpose(1, 0, 2).reshape(128, E)
        ).astype(SCORE_NP)

    wqq = pack_w(np.concatenate([Wq, Wq], axis=1))
    wkv = pack_w(np.concatenate([Wk, wv8], axis=1))
    wvk = pack_w(np.concatenate([wv8, Wk], axis=1))
    in_maps = []
    for c in range(NCORES):
        b, h = divmod(c, 2)
        xb = x[b]
        rot = np.concatenate([xb[h * TH : (h + 1) * TH], xb[(1 - h) * TH : (2 - h) * TH]])
        xT = np.ascontiguousarray(rot.T).astype(SCORE_NP)  # [E, T]
        in_maps.append({"xT": xT, "wqq": wqq, "wkv": wkv, "wvk": wvk})
    return in_maps


def run(in_maps, trace=False, **kwargs):
    global LAST_RESULT
    nc = _get_nc()
    LAST_RESULT = run_bass_kernel_spmd(
        nc, in_maps, core_ids=list(range(NCORES)), trace=trace, **kwargs
    )
    return LAST_RESULT


def assemble(results):
    out = np.empty((B, T, D), dtype=np.float32)
    for c in range(NCORES):
        b, h = divmod(c, 2)
        ot = results[c]["outT"]  # [D+1, TH]: rows 0:D numerator/8, row D sums
        out[b, h * TH : (h + 1) * TH] = (ot[0:D] / ot[D : D + 1]).T
    return out


def kernel(x, Wq, Wk, Wv):
    res = run(make_in_maps(x, Wq, Wk, Wv), trace=bool(os.environ.get("BASS_TRACE")))
    return assemble(res.results)


# revision 81
# speedup vs baseline: 1.0123x; 1.0123x over previous
"""Single-head attention (B=4, T=4096, E=1024, D=64) on 8 TRN2 NeuronCores.

Sharding: data-parallel over (batch, query-half): core c -> batch c//2,
query half c%2.  Each core receives the full x[b] pre-transposed on the
host, with rows rotated so its OWN query half always occupies columns
0:2048 (keeps the SPMD graph identical across cores; attention is
permutation-invariant over keys).

Per-core pipeline (score-chain matmuls in fp16):
  1. Projections (PE, stationary weights) per 512-token chunk:
     own chunks also run [Wq|Wq] (Q^T duplicated to BOTH partition
     halves -- output duplication is free, matmul cost = moving cols);
     even chunks run [Wk|Wv/8] (K^T -> rows 0:64, V^T -> rows 64:128),
     odd chunks run [Wv/8|Wk] (V^T -> rows 0:64, K^T -> rows 64:128).
     The 1/sqrt(D) is folded into Wv.
  2. V^T -> V' = [V | ones] strips via PE-transpose (the ones column
     makes P @ V' also emit softmax row sums).
  3. Scores are ROW-PACKED on the PE: pair p = (tile from an even
     chunk at PE rows 0:63, tile from an odd chunk at rows 64:127).
     The two 64-contraction matmuls run concurrently on disjoint
     row-groups (tile_position comes from the operands' base
     partitions), doubling score throughput.  exp on ScalarE
     (PSUM -> SBUF bf16), O^T += V'.T @ P^T into [65, 1024] PSUM.
  4. O^T (unnormalized, with the sums row) is DMA'd straight to DRAM;
     the host does the transpose + divide (free).

Softmax runs without max-subtraction: scores are ~N(0, 64) so |s| << 88
(fp32 exp overflow); the reference's max-subtraction is a no-op.
"""

import os
import sys
from collections import deque

import numpy as np

_TRN_REPO = "/opt/trn_rl_repo"
if _TRN_REPO not in sys.path:
    sys.path.insert(0, _TRN_REPO)

import concourse.bass as bass  # noqa: E402
import concourse.mybir as mybir  # noqa: E402
import concourse.tile as tile  # noqa: E402
from concourse import bacc  # noqa: E402
from concourse.bass_utils import run_bass_kernel_spmd  # noqa: E402

F32 = mybir.dt.float32
F16 = mybir.dt.float16
BF16 = mybir.dt.bfloat16

B, T, E, D = 4, 4096, 1024, 64
TH = T // 2  # queries per core
NCORES = 8
QPASS = 1024  # queries per PSUM pass
NMM = 512  # matmul moving free dim (one fp32 PSUM bank)
NKT = T // 128  # 32 key tiles of 128
NPAIR = NKT // 2  # 16 row-packed tile pairs
EK = E // 128  # 8 contraction tiles for projections
CHUNK = 512  # projection chunk (tokens)
NCH = T // CHUNK  # 8 chunks

SCORE_DT = F16
SCORE_NP = np.float16
PV_DT = BF16  # P = exp(S) reaches ~1e20: needs bf16 range


def _tile_a(p):  # even-chunk tile of pair p (PE rows 0:64)
    return 8 * (p // 4) + p % 4


def _tile_b(p):  # odd-chunk tile of pair p (PE rows 64:128)
    return _tile_a(p) + 4


def _build_nc() -> bass.Bass:
    nc = bacc.Bacc(
        "TRN2",
        target_bir_lowering=False,
        debug=False,
        num_devices=NCORES,
    )
    xT_d = nc.dram_tensor("xT", [E, T], SCORE_DT, kind="ExternalInput")
    # weights arrive partition-major ([128, E], e-blocks along the free
    # dim) so the load is 128 x 2KB descriptors instead of 1024 x 256B
    wqq_d = nc.dram_tensor("wqq", [128, E], SCORE_DT, kind="ExternalInput")
    wkv_d = nc.dram_tensor("wkv", [128, E], SCORE_DT, kind="ExternalInput")
    wvk_d = nc.dram_tensor("wvk", [128, E], SCORE_DT, kind="ExternalInput")
    out_d = nc.dram_tensor("outT", [D + 1, TH], F32, kind="ExternalOutput")

    with tile.TileContext(nc) as tc:
        with (
            tc.tile_pool(name="consts", bufs=1) as consts,
            tc.tile_pool(name="big", bufs=1) as big,
            tc.tile_pool(name="pt", bufs=8) as ptpool,
            tc.tile_pool(name="osb", bufs=2) as osbpool,
            tc.tile_pool(name="auxp", bufs=2, space="PSUM") as auxp,
            tc.tile_pool(name="stp", bufs=2, space="PSUM") as stp,
            tc.tile_pool(name="otp", bufs=1, space="PSUM") as otp,
        ):
            # identity for the V^T -> V PE-transposes, on both partition
            # halves (even chunks park V^T at rows 64:128, odd at 0:64);
            # emitted first so the gpsimd queue is clear for xT loads
            identB = consts.tile([128, 64], PV_DT, tag="identB")
            from concourse.masks import make_identity

            nc.gpsimd.memset(identB[:], 0.0)
            make_identity(nc, identB[0:64, 0:64], nomemset=True)
            make_identity(nc, identB[64:128, 0:64], nomemset=True)

            # weights go on the scalar queue: it is idle this early, and
            # these three transfers drain long before the first exp
            wqq = consts.tile([128, E], SCORE_DT, tag="wqq")
            wkv = consts.tile([128, E], SCORE_DT, tag="wkv")
            wvk = consts.tile([128, E], SCORE_DT, tag="wvk")
            for w_sb, w_d in ((wqq, wqq_d), (wkv, wkv_d), (wvk, wvk_d)):
                nc.scalar.dma_start(w_sb[:], w_d[:, :])

            # V' strip: 32 tiles of [128 keys, 64 V cols + 1 ones col],
            # padded to stride 128.  Only the ones columns need init
            # (cols 0:64 are overwritten, 65:128 never read).
            vprime = consts.tile([128, NKT * 128], PV_DT, tag="vprime")
            nc.vector.memset(
                vprime[:].rearrange("p (k c) -> p k c", c=128)[:, :, 64:65], 1.0
            )

            # (no explicit exp-table prewarm: the auto-inserted table
            # load before the first real exp overlaps the proj phase; an
            # early ACTIVATE gets hoisted ahead of the weight DMAs on
            # the scalar queue and delays the first projection)

            # HAM warmup: keep the PE busy from t~0 until the first
            # projection chunk lands so real work runs at 2.4 GHz
            # warm PSUM lives in the otp pool slot (free until pass 0's
            # accumulator is allocated) so fillers never clobber the
            # rotating aux slots used by in-flight projections
            warm = consts.tile([128, 256], SCORE_DT, tag="warm")
            nc.vector.memset(warm[:], 0.0)
            wps = otp.tile([128, 256], F32, tag="ot", name="wps")
            # 16 x 256 cols ~= 3.4us at cold clock: bridges from t~8us to
            # the first projection AND satisfies the HAM SHORT window so
            # the projections run at 2.4 GHz from the start
            for _ in range(16):
                nc.tensor.matmul(wps[:], warm[:, 0:128], warm[:], start=True, stop=True)

            q2 = big.tile([128, TH], SCORE_DT, tag="q2")  # Q^T on both halves
            k2 = big.tile([128, NPAIR * 128], SCORE_DT, tag="k2")
            vt = big.tile([128, T], PV_DT, tag="vt")

            # ---- x^T loads: chunk-major so chunk c unlocks after 8 tiles ----
            # the scalar queue is deliberately NOT used for input DMAs:
            # its semaphore waits would block the ACTIVATEs (exp) queued
            # behind them on the same engine
            xts = [None] * NCH
            engs = (nc.sync, nc.gpsimd)
            ei = 0
            for c in range(NCH):
                xt = big.tile([128, EK * CHUNK], SCORE_DT, tag=f"xt{c}", name=f"xt{c}")
                for e in range(EK):
                    engs[ei % 2].dma_start(
                        xt[:, e * CHUNK : (e + 1) * CHUNK],
                        xT_d[e * 128 : (e + 1) * 128, c * CHUNK : (c + 1) * CHUNK],
                    )
                    ei += 1
                xts[c] = xt

            # ---- projection emitters ----
            def emit_proj_Q(c):
                xt = xts[c]
                pa = auxp.tile([128, CHUNK], F32, tag="aux", name=f"pa{c}")
                for e in range(EK):
                    nc.tensor.matmul(
                        pa[:],
                        wqq[:, e * 128 : (e + 1) * 128],
                        xt[:, e * CHUNK : (e + 1) * CHUNK],
                        start=(e == 0),
                        stop=(e == EK - 1),
                    )
                nc.vector.tensor_copy(q2[:, c * CHUNK : (c + 1) * CHUNK], pa[:])

            def emit_proj_K(c):
                even = c % 2 == 0
                xt = xts[c]
                wkvx = wkv if even else wvk
                krows, vrows = (
                    (slice(0, 64), slice(64, 128))
                    if even
                    else (slice(64, 128), slice(0, 64))
                )
                pb = auxp.tile([128, CHUNK], F32, tag="aux", name=f"pb{c}")
                for e in range(EK):
                    nc.tensor.matmul(
                        pb[:],
                        wkvx[:, e * 128 : (e + 1) * 128],
                        xt[:, e * CHUNK : (e + 1) * CHUNK],
                        start=(e == 0),
                        stop=(e == EK - 1),
                    )
                nc.vector.tensor_copy(
                    k2[krows, (c // 2) * CHUNK : (c // 2 + 1) * CHUNK],
                    pb[krows, :],
                )
                nc.vector.tensor_copy(
                    vt[vrows, c * CHUNK : (c + 1) * CHUNK], pb[vrows, :]
                )
                for j in range(4):
                    kb = 4 * c + j
                    tp = auxp.tile([128, D], PV_DT, tag="aux", name=f"tp{kb}")
                    nc.tensor.transpose(
                        tp[:],
                        vt[vrows, kb * 128 : (kb + 1) * 128],
                        identB[vrows, 0:64],
                    )
                    nc.vector.tensor_copy(
                        vprime[:, kb * 128 : kb * 128 + D], tp[:]
                    )

            # chunks 0,1: Q and K sweeps interleaved per e-tile so the
            # projections trail the DMA arrivals by ~2 matmuls
            for c in (0, 1):
                xt = xts[c]
                wkvx = wkv if c % 2 == 0 else wvk
                # Q accumulators borrow the (still unused) score-pool
                # slots so all four head accumulators coexist without
                # PSUM rotation conflicts
                pa = stp.tile([128, CHUNK], F32, tag="st", name=f"pa{c}")
                pb = auxp.tile([128, CHUNK], F32, tag="aux", name=f"pb{c}")
                for e in range(EK):
                    nc.tensor.matmul(
                        pa[:],
                        wqq[:, e * 128 : (e + 1) * 128],
                        xt[:, e * CHUNK : (e + 1) * CHUNK],
                        start=(e == 0),
                        stop=(e == EK - 1),
                    )
                    nc.tensor.matmul(
                        pb[:],
                        wkvx[:, e * 128 : (e + 1) * 128],
                        xt[:, e * CHUNK : (e + 1) * CHUNK],
                        start=(e == 0),
                        stop=(e == EK - 1),
                    )

                nc.vector.tensor_copy(q2[:, c * CHUNK : (c + 1) * CHUNK], pa[:])
                krows, vrows = (
                    (slice(0, 64), slice(64, 128))
                    if c % 2 == 0
                    else (slice(64, 128), slice(0, 64))
                )
                nc.vector.tensor_copy(
                    k2[krows, (c // 2) * CHUNK : (c // 2 + 1) * CHUNK], pb[krows, :]
                )
                nc.vector.tensor_copy(
                    vt[vrows, c * CHUNK : (c + 1) * CHUNK], pb[vrows, :]
                )
                for j in range(4):
                    kb = 4 * c + j
                    tp = auxp.tile([128, D], PV_DT, tag="aux", name=f"tp{kb}")
                    nc.tensor.transpose(
                        tp[:],
                        vt[vrows, kb * 128 : (kb + 1) * 128],
                        identB[vrows, 0:64],
                    )
                    nc.vector.tensor_copy(vprime[:, kb * 128 : kb * 128 + D], tp[:])

            # remaining projections: full K units (dense PE blocks keep
            # the HAM clock warm), pair 0 left clean so the first exps
            # fire as early as possible; Q-sweeps of chunks 2,3 (pass-1
            # queries) deferred to the end of pass 0
            chunk_sched = {
                1: [("K", 2)],
                2: [("K", 3)],
                4: [("K", 4)],
                6: [("K", 5)],
                8: [("K", 6)],
                10: [("K", 7)],
                12: [("Q", 2)],
                13: [("Q", 3)],
            }

            # ---- attention passes ----
            for qp in range(TH // QPASS):
                q0 = qp * QPASS
                ot = otp.tile([D + 1, QPASS], F32, tag="ot")
                pending_av = deque()
                n_av = [0]

                def emit_av(avpt, avkt):
                    for qc in range(0, QPASS, NMM):
                        nc.tensor.matmul(
                            ot[:, qc : qc + NMM],
                            vprime[:, avkt * 128 : avkt * 128 + D + 1],
                            avpt[:, qc : qc + NMM],
                            start=(n_av[0] == 0),
                            stop=(n_av[0] == NKT - 1),
                        )
                    n_av[0] += 1

                for p in range(NPAIR):
                    ka, kb = _tile_a(p), _tile_b(p)
                    st_e = stp.tile([128, QPASS], F32, tag="st", name=f"se{qp}_{p}")
                    st_o = stp.tile([128, QPASS], F32, tag="st", name=f"so{qp}_{p}")
                    for qc in range(0, QPASS, NMM):
                        nc.tensor.matmul(
                            st_e[:, qc : qc + NMM],
                            k2[0:64, p * 128 : (p + 1) * 128],
                            q2[0:64, q0 + qc : q0 + qc + NMM],
                            start=True,
                            stop=True,
                        )
                        nc.tensor.matmul(
                            st_o[:, qc : qc + NMM],
                            k2[64:128, p * 128 : (p + 1) * 128],
                            q2[64:128, q0 + qc : q0 + qc + NMM],
                            start=True,
                            stop=True,
                        )
                    pt_e = ptpool.tile([128, QPASS], PV_DT, tag="pt", name=f"pe{qp}_{p}")
                    nc.scalar.activation(
                        pt_e[:], st_e[:], mybir.ActivationFunctionType.Exp
                    )
                    pt_o = ptpool.tile([128, QPASS], PV_DT, tag="pt", name=f"po{qp}_{p}")
                    nc.scalar.activation(
                        pt_o[:], st_o[:], mybir.ActivationFunctionType.Exp
                    )
                    pending_av.append((pt_e, ka))
                    pending_av.append((pt_o, kb))
                    # shallower backlog near the pass end shortens the tail
                    depth = 3 if p < NPAIR - 2 else (1 if p < NPAIR - 1 else 0)
                    while len(pending_av) > depth:
                        emit_av(*pending_av.popleft())
                    # projection units go AFTER the pair's scores so the
                    # exp chain is never pushed out by projection matmuls
                    if qp == 0 and p in chunk_sched:
                        for kind, c in chunk_sched[p]:
                            (emit_proj_K if kind == "K" else emit_proj_Q)(c)
                while pending_av:
                    emit_av(*pending_av.popleft())

                # ship the unnormalized O^T (plus the sums row) straight
                # out; the host does transpose + divide.  Two halves so
                # the copy of half 0 overlaps the final AVs of half 1.
                osb = osbpool.tile([D + 1, QPASS], F32, tag="osb")
                for h in (0, 1):
                    sl = slice(h * (QPASS // 2), (h + 1) * (QPASS // 2))
                    nc.vector.tensor_copy(osb[:, sl], ot[:, sl])
                    nc.sync.dma_start(out_d[:, q0 + h * (QPASS // 2) : q0 + (h + 1) * (QPASS // 2)], osb[:, sl])

    _elide_redundant_ldweights(nc)
    nc.compile()
    return nc


def _elide_redundant_ldweights(nc):
    """Drop an InstLdweights whose stationary AP is identical to the
    previous one with only plain matmuls between (the legalizer emits one
    load per matmul; consecutive same-weights loads are dead)."""
    removed = 0
    for blk in nc.main_func.blocks:
        last_key = {}  # row-group (base partition span) -> AP key
        keep = []
        for inst in blk.instructions:
            if isinstance(inst, mybir.InstLdweights):
                si = inst.sync_info
                clean = si is None or (not si.on_wait and not si.on_update)
                ap = inst.ins[0]
                key = repr(ap)
                bap = getattr(ap, "bass_ap", None)
                part0 = psz = None
                if bap is not None:
                    try:
                        part0 = bap.base_partition()
                        psz = bap.partition_size()
                    except Exception:
                        part0 = psz = None
                grp = (part0, psz)
                full = psz is None or part0 is None or psz > 64
                if clean and part0 is not None and last_key.get(grp) == key:
                    removed += 1
                    continue
                if full:
                    last_key.clear()
                    if part0 is not None:
                        last_key[grp] = key
                else:
                    # a load into one row-group leaves other groups intact
                    last_key = {
                        g: k
                        for g, k in last_key.items()
                        if g[0] + (g[1] or 128) <= part0
                        or part0 + (psz or 128) <= g[0]
                    }
                    last_key[grp] = key
                keep.append(inst)
                continue
            if getattr(inst, "engine", None) == mybir.EngineType.PE:
                if not (
                    isinstance(inst, mybir.InstMatmult)
                    and not getattr(inst, "is_transpose", False)
                ):
                    last_key = {}
            keep.append(inst)
        blk.instructions[:] = keep
    return removed


_NC_CACHE = None
LAST_RESULT = None


def _get_nc():
    global _NC_CACHE
    if _NC_CACHE is None:
        _NC_CACHE = _build_nc()
    return _NC_CACHE


def make_in_maps(x, Wq, Wk, Wv):
    x = np.asarray(x, dtype=np.float32)
    Wq = np.asarray(Wq, dtype=np.float32)
    Wk = np.asarray(Wk, dtype=np.float32)
    Wv = np.asarray(Wv, dtype=np.float32)
    wv8 = Wv / np.sqrt(np.float32(D))

    def pack_w(wcat):  # [E, 128] -> [128, E] partition-major, e-blocks
        return np.ascontiguousarray(
            wcat.reshape(EK, 128, 128).transpose(1, 0, 2).reshape(128, E)
        ).astype(SCORE_NP)

    wqq = pack_w(np.concatenate([Wq, Wq], axis=1))
    wkv = pack_w(np.concatenate([Wk, wv8], axis=1))
    wvk = pack_w(np.concatenate([wv8, Wk], axis=1))
    in_maps = []
    for c in range(NCORES):
        b, h = divmod(c, 2)
        xb = x[b]
        rot = np.concatenate([xb[h * TH : (h + 1) * TH], xb[(1 - h) * TH : (2 - h) * TH]])
        xT = np.ascontiguousarray(rot.T).astype(SCORE_NP)  # [E, T]
        in_maps.append({"xT": xT, "wqq": wqq, "wkv": wkv, "wvk": wvk})
    return in_maps


def run(in_maps, trace=False, **kwargs):
    global LAST_RESULT
    nc = _get_nc()
    LAST_RESULT = run_bass_kernel_spmd(
        nc, in_maps, core_ids=list(range(NCORES)), trace=trace, **kwargs
    )
    return LAST_RESULT


def assemble(results):
    out = np.empty((B, T, D), dtype=np.float32)
    for c in range(NCORES):
        b, h = divmod(c, 2)
        ot = results[c]["outT"]  # [D+1, TH]: rows 0:D numerator/8, row D sums
        out[b, h * TH : (h + 1) * TH] = (ot[0:D] / ot[D : D + 1]).T
    return out


def kernel(x, Wq, Wk, Wv):
    res = run(make_in_maps(x, Wq, Wk, Wv), trace=bool(os.environ.get("BASS_TRACE")))
    return assemble(res.results)
